# revision 30
# baseline (speedup 1.0000x reference)
"""MoE top-2 routing kernel for 8 Trainium2 NeuronCores — expert-parallel.

Problem: x[2,4096,1024] tokens, 8 experts W[8,1024,1024]+b[8,1024],
top-2 expert indices + gate weights per token.
out[t] = sum_k gate[t,k] * (x[t] @ W[idx[t,k]] + b[idx[t,k]])

Strategy (expert-parallel, host-side dispatch):
- E == n_cores == 8: core e owns expert e. The host routes: dedup the two
  (expert, gate) entries per token (same-expert duplicates merge, gates
  summed), groups entries by expert, and builds per-core inputs:
    xg   [128, Tmax*1024] fp16 — the expert's token rows, PE-transposed
         ([p, t*1024 + c*128 + m] = x[row t*128+m, c*128+p]), zero-padded
         to Tmax 128-row tiles.
    w    [128, 8*1024]    fp16 — W_e PE layout ([p, c*1024+f] = W_e[c*128+p, f])
    g    [128, Tmax]      f32  — gate per dispatch row ([m, t] = gate row t*128+m)
- Device: per 128-row tile, 16 accumulating fp16 matmuls (8 K-chunks x 2
  PSUM halves) -> DVE gate-scale (f32 PSUM -> fp16) -> contiguous DMA
  store. No gather/scatter ucode, no replicated W, no bias matmuls.
- Host combine: out[t] = Y[slot0[t]] + Y[slot1[t]] + g0*b[e0] + g1*b[e1]
  (slot1 -> zero row for merged/single-entry tokens); bias exact in f32.
- Load balance: per-expert entry counts are multinomial(~1920 +/- 40);
  Tmax = max_e ceil(n_e/128) == ceil(total_tiles/8) for typical draws, so
  expert-parallel matches the best possible row balance.
"""

import os
import sys

import numpy as np

for _p in ("/opt/trn_rl_repo", os.path.expanduser("~/.axon_site/_ro/trn_rl_repo")):
    if os.path.isdir(_p) and _p not in sys.path:
        sys.path.insert(0, _p)

B, S, D, E, K = 2, 4096, 1024, 8, 2
N_CORES = 8
TOKENS = B * S
P = 128
DCHUNKS = D // P  # 8
FH = 512  # psum bank half of D
NH = D // FH  # 2
WARMUP = 5


def _route(top_k_indices, expert_weights):
    """Dedup + group entries by expert.

    Returns (toks, gs, n_e, Tmax, cum, slot):
      toks/gs: token id and gate per dispatch entry, sorted by expert
      n_e[e]: entry count of expert e; cum[e]: its offset in the sort
      Tmax: per-core tile count = max_e ceil(n_e/128)
      slot[t, 0:2]: global padded-Y row of token t's entries (ZROW = none)
    """
    idx = np.asarray(top_k_indices).reshape(-1, K).astype(np.int64)
    gw = np.asarray(expert_weights).reshape(-1, K).astype(np.float32)
    dup = idx[:, 0] == idx[:, 1]
    g0 = np.where(dup, gw[:, 0] + gw[:, 1], gw[:, 0])
    keep = ~dup
    toks = np.concatenate([np.arange(TOKENS), np.arange(TOKENS)[keep]])
    exps = np.concatenate([idx[:, 0], idx[keep, 1]])
    gs = np.concatenate([g0, gw[keep, 1]])
    order = np.argsort(exps, kind="stable")
    toks, exps, gs = toks[order], exps[order], gs[order]
    n_e = np.bincount(exps, minlength=E)
    Tmax = max(1, int(np.max(-(-n_e // P))))
    cum = np.concatenate([[0], np.cumsum(n_e)])[:E]
    pos_in_e = np.arange(toks.size) - cum[exps]
    yrow = exps * (Tmax * P) + pos_in_e
    inv = np.empty_like(order)
    inv[order] = np.arange(order.size)
    ZROW = E * Tmax * P
    slot = np.full((TOKENS, 2), ZROW, np.int64)
    slot[:, 0] = yrow[inv[:TOKENS]]
    slot[keep, 1] = yrow[inv[TOKENS:]]
    return toks, gs, n_e, Tmax, cum, slot


def _prep_inputs(x, top_k_indices, expert_weights, W):
    toks, gs, n_e, Tmax, cum, slot = _route(top_k_indices, expert_weights)
    x_flat = np.asarray(x, np.float32).reshape(TOKENS, D).astype(np.float16)
    W16 = np.asarray(W, np.float32).astype(np.float16)
    in_maps = []
    for e in range(E):
        n = int(n_e[e])
        seg = slice(cum[e], cum[e] + n)
        xr = np.zeros((Tmax * P, D), np.float16)
        xr[:n] = x_flat[toks[seg]]
        # [t*128+m, c*128+p] -> [p, t, c, m]
        xg = np.ascontiguousarray(
            xr.reshape(Tmax, P, DCHUNKS, P).transpose(3, 0, 2, 1)
        ).reshape(P, Tmax * D)
        gr = np.zeros(Tmax * P, np.float32)
        gr[:n] = gs[seg]
        g_sb = np.ascontiguousarray(gr.reshape(Tmax, P).T)
        w_hw = np.ascontiguousarray(
            W16[e].reshape(DCHUNKS, P, D).transpose(1, 0, 2)
        ).reshape(P, DCHUNKS * D)
        in_maps.append({"xg": xg, "w": w_hw, "g": g_sb})
    return in_maps, Tmax, slot


def _build_program(Tmax):
    import concourse.tile as tile
    from concourse import bacc, mybir

    fp16 = mybir.dt.float16
    f32 = mybir.dt.float32

    nc = bacc.Bacc("TRN2", target_bir_lowering=False, debug=False)
    xg_d = nc.dram_tensor("xg", [P, Tmax * D], fp16, kind="ExternalInput").ap()
    w_d = nc.dram_tensor("w", [P, DCHUNKS * D], fp16, kind="ExternalInput").ap()
    g_d = nc.dram_tensor("g", [P, Tmax], f32, kind="ExternalInput").ap()
    y_d = nc.dram_tensor("y", [Tmax * P, D], fp16, kind="ExternalOutput").ap()

    with tile.TileContext(nc) as tc:
        with (
            tc.tile_pool(name="const", bufs=1) as cpool,
            tc.tile_pool(name="xp", bufs=Tmax) as xpool,
            tc.tile_pool(name="yp", bufs=3) as ypool,
            tc.tile_pool(name="ps", bufs=4, space="PSUM") as pspool,
        ):
            # Two HWDGE queues (sync SP + scalar ACT), each ~130 GB/s when both
            # stream and limited to 4 in-flight transfers. The head is
            # bandwidth-bound: W (2 MiB) + xg0/xg1 must land before tiles 0-1
            # finish, so supply is issued in half-chunk (128 KB) transfers,
            # dealt across the queues in consumption order; tiles 0-1 are
            # computed chunk-major so the PE rides the arrival stream.
            xgs = [xpool.tile([P, D], fp16, tag="xg", name="xg") for t in range(Tmax)]
            whs = [
                [cpool.tile([P, FH], fp16, name=f"w{c}h{h}") for h in range(NH)]
                for c in range(DCHUNKS)
            ]
            g_sb = cpool.tile([P, Tmax], f32)

            def wsl(c, h):  # rhs AP for chunk c, psum half h
                return whs[c][h][:]

            def kick_xg_half(ring, t, a):
                ring.dma_start(
                    xgs[t][:, a * FH : (a + 1) * FH],
                    xg_d[:, t * D + a * FH : t * D + (a + 1) * FH],
                )

            def kick_w(ring, c, h):
                ring.dma_start(
                    whs[c][h][:], w_d[:, c * D + h * FH : c * D + (h + 1) * FH]
                )

            # Supply deal-out in need order (tiles 0-2 chunk-major consumption):
            # all pieces as 128 KB half-chunks, alternating queues down the
            # need list so each chunk's halves arrive adjacently and W leads.
            # NI=3 leaves PSUM buffer 3 free, so the first serial tile never
            # waits on a DVE drain (any PE idle gap >100ns resets the p-state
            # ramp and costs ~3us of half-rate matmuls).
            NI = min(3, Tmax)  # tiles interleaved chunk-major in the head
            need = [
                ("w", 0, 0), ("w", 0, 1), ("xga", 0), ("xga", 1),
                ("w", 1, 0), ("xga", 2), ("w", 1, 1),
                ("w", 2, 0), ("w", 2, 1), ("w", 3, 0), ("w", 3, 1),
                ("w", 4, 0), ("xgb", 0), ("w", 4, 1), ("xgb", 1),
                ("w", 5, 0), ("xgb", 2), ("w", 5, 1),
                ("w", 6, 0), ("w", 6, 1), ("w", 7, 0), ("w", 7, 1),
                ("g",),
            ]
            for i, item in enumerate(need):
                ring = nc.sync if i % 2 == 0 else nc.scalar
                if item[0] == "w":
                    kick_w(ring, item[1], item[2])
                elif item[0] == "xga":
                    kick_xg_half(ring, item[1], 0)
                elif item[0] == "xgb":
                    kick_xg_half(ring, item[1], 1)
                else:
                    ring.dma_start(g_sb[:], g_d[:])
            for t in range(NI, Tmax):
                nc.scalar.dma_start(xgs[t][:], xg_d[:, t * D : (t + 1) * D])

            # PSUM tiles for the chunk-major head (all 8 banks; the pool then
            # rotates through the same 4 buffers for the serial tiles).
            psNI = [pspool.tile([P, D], f32, tag="ps", name="ps") for _ in range(NI)]

            # PE warmup on FULL-SIZE dummy matmuls (128 contraction x 512 out):
            # the clock ramp is utilization-driven, so 1-row warmups leave the
            # PE at the 1.2 GHz mid p-state and the first ~4us of real matmuls
            # run at half rate. Full-size warmups ramp to 2.4 GHz while the
            # first supply transfers are still in flight. Targets ps0
            # (complete start/stop groups, reset by the first real matmul).
            wl = cpool.tile([P, P], fp16)
            nc.vector.memset(wl[:], 1.0)
            wr = cpool.tile([P, FH], fp16)
            nc.vector.memset(wr[:], 1.0)
            for _ in range(WARMUP):
                nc.tensor.matmul(
                    psNI[0][:, 0:FH], wl[:], wr[:], start=True, stop=True
                )

            def scale_store(t, ps, last):
                """PSUM -> fp16 gate-scale -> DRAM store. The final tile's two
                halves run in PARALLEL on vector (DVE) and scalar (ACT) with
                separate queue stores, halving the drain critical path."""
                y_sb = ypool.tile([P, D], fp16, tag="y", name="y_sb")
                if not last:
                    nc.vector.tensor_scalar_mul(y_sb[:], ps[:], g_sb[:, t : t + 1])
                    nc.sync.dma_start(y_d[t * P : (t + 1) * P, :], y_sb[:])
                    return
                nc.vector.tensor_scalar_mul(
                    y_sb[:, 0:FH], ps[:, 0:FH], g_sb[:, t : t + 1]
                )
                nc.scalar.mul(y_sb[:, FH:D], ps[:, FH:D], g_sb[:, t : t + 1])
                nc.sync.dma_start(y_d[t * P : (t + 1) * P, 0:FH], y_sb[:, 0:FH])
                nc.scalar.dma_start(y_d[t * P : (t + 1) * P, FH:D], y_sb[:, FH:D])

            # Tiles 0..NI-1 chunk-major: each W half-chunk is consumed by all
            # NI tiles as soon as it lands; with NI*2*213ns of work per chunk
            # the PE outpaces the supply stream and never goes idle.
            for c in range(DCHUNKS):
                for tt in range(NI):
                    for h in range(NH):
                        nc.tensor.matmul(
                            psNI[tt][:, h * FH : (h + 1) * FH],
                            xgs[tt][:, c * P : (c + 1) * P],
                            wsl(c, h),
                            start=(c == 0),
                            stop=(c == DCHUNKS - 1),
                        )
            # Head tiles' DVE in halves: releases PSUM banks to tile NI (which
            # reuses buffer 0) half a DVE pass sooner.
            for tt in range(NI):
                y_sb = ypool.tile([P, D], fp16, tag="y", name="y_sb")
                for h in range(NH):
                    sl = slice(h * FH, (h + 1) * FH)
                    nc.vector.tensor_scalar_mul(
                        y_sb[:, sl], psNI[tt][:, sl], g_sb[:, tt : tt + 1]
                    )
                nc.sync.dma_start(y_d[tt * P : (tt + 1) * P, :], y_sb[:])

            for t in range(NI, Tmax):
                ps = pspool.tile([P, D], f32, tag="ps", name="ps")
                for c in range(DCHUNKS):
                    lhsT = xgs[t][:, c * P : (c + 1) * P]
                    for h in range(NH):
                        nc.tensor.matmul(
                            ps[:, h * FH : (h + 1) * FH],
                            lhsT,
                            wsl(c, h),
                            start=(c == 0),
                            stop=(c == DCHUNKS - 1),
                        )
                scale_store(t, ps, last=(t == Tmax - 1))
    nc.compile()
    return nc


def kernel(x, expert_weights, top_k_indices, W, b):
    from concourse.bass_utils import run_bass_kernel_spmd

    in_maps, Tmax, slot = _prep_inputs(x, top_k_indices, expert_weights, W)
    nc = _build_program(Tmax)
    res = run_bass_kernel_spmd(
        nc,
        in_maps,
        core_ids=list(range(N_CORES)),
        trace=bool(int(os.environ.get("KERNEL_TRACE", "0"))),
    )
    Y = np.concatenate(
        [res.results[e]["y"] for e in range(E)] + [np.zeros((1, D), np.float16)]
    ).astype(np.float32)
    idx = np.asarray(top_k_indices).reshape(-1, K)
    gw = np.asarray(expert_weights, np.float32).reshape(-1, K)
    b32 = np.asarray(b, np.float32)
    out = Y[slot[:, 0]] + Y[slot[:, 1]]
    out += gw[:, 0, None] * b32[idx[:, 0]]
    out += gw[:, 1, None] * b32[idx[:, 1]]
    if bool(int(os.environ.get("KERNEL_TRACE", "0"))):
        kernel.last_results = res
    return np.ascontiguousarray(out.reshape(B, S, D))


# revision 31
# speedup vs baseline: 1.2155x; 1.2155x over previous
"""MoE top-2 routing kernel for 8 Trainium2 NeuronCores — expert-parallel.

Problem: x[2,4096,1024] tokens, 8 experts W[8,1024,1024]+b[8,1024],
top-2 expert indices + gate weights per token.
out[t] = sum_k gate[t,k] * (x[t] @ W[idx[t,k]] + b[idx[t,k]])

Strategy (expert-parallel, host-side dispatch):
- E == n_cores == 8: core e owns expert e. The host routes: dedup the two
  (expert, gate) entries per token (same-expert duplicates merge, gates
  summed), groups entries by expert, and builds per-core inputs:
    xg   [128, Tmax*1024] fp16 — the expert's token rows, PE-transposed
         ([p, t*1024 + c*128 + m] = x[row t*128+m, c*128+p]), zero-padded
         to Tmax 128-row tiles.
    w    [128, 8*1024]    fp16 — W_e PE layout ([p, c*1024+f] = W_e[c*128+p, f])
    g    [128, Tmax]      f32  — gate per dispatch row ([m, t] = gate row t*128+m)
- Device: per 128-row tile, 16 accumulating fp16 matmuls (8 K-chunks x 2
  PSUM halves) -> DVE gate-scale (f32 PSUM -> fp16) -> contiguous DMA
  store. No gather/scatter ucode, no replicated W, no bias matmuls.
- Host combine: out[t] = Y[slot0[t]] + Y[slot1[t]] + g0*b[e0] + g1*b[e1]
  (slot1 -> zero row for merged/single-entry tokens); bias exact in f32.
- Load balance: per-expert entry counts are multinomial(~1920 +/- 40);
  Tmax = max_e ceil(n_e/128) == ceil(total_tiles/8) for typical draws, so
  expert-parallel matches the best possible row balance.
"""

import os
import sys

import numpy as np

for _p in ("/opt/trn_rl_repo", os.path.expanduser("~/.axon_site/_ro/trn_rl_repo")):
    if os.path.isdir(_p) and _p not in sys.path:
        sys.path.insert(0, _p)

B, S, D, E, K = 2, 4096, 1024, 8, 2
N_CORES = 8
TOKENS = B * S
P = 128
DCHUNKS = D // P  # 8
FH = 512  # psum bank half of D
NH = D // FH  # 2
WARMUP = 9


def _route(top_k_indices, expert_weights):
    """Dedup + group entries by expert.

    Returns (toks, gs, n_e, Tmax, cum, slot):
      toks/gs: token id and gate per dispatch entry, sorted by expert
      n_e[e]: entry count of expert e; cum[e]: its offset in the sort
      Tmax: per-core tile count = max_e ceil(n_e/128)
      slot[t, 0:2]: global padded-Y row of token t's entries (ZROW = none)
    """
    idx = np.asarray(top_k_indices).reshape(-1, K).astype(np.int64)
    gw = np.asarray(expert_weights).reshape(-1, K).astype(np.float32)
    dup = idx[:, 0] == idx[:, 1]
    g0 = np.where(dup, gw[:, 0] + gw[:, 1], gw[:, 0])
    keep = ~dup
    toks = np.concatenate([np.arange(TOKENS), np.arange(TOKENS)[keep]])
    exps = np.concatenate([idx[:, 0], idx[keep, 1]])
    gs = np.concatenate([g0, gw[keep, 1]])
    order = np.argsort(exps, kind="stable")
    toks, exps, gs = toks[order], exps[order], gs[order]
    n_e = np.bincount(exps, minlength=E)
    Tmax = max(1, int(np.max(-(-n_e // P))))
    cum = np.concatenate([[0], np.cumsum(n_e)])[:E]
    pos_in_e = np.arange(toks.size) - cum[exps]
    yrow = exps * (Tmax * P) + pos_in_e
    inv = np.empty_like(order)
    inv[order] = np.arange(order.size)
    ZROW = E * Tmax * P
    slot = np.full((TOKENS, 2), ZROW, np.int64)
    slot[:, 0] = yrow[inv[:TOKENS]]
    slot[keep, 1] = yrow[inv[TOKENS:]]
    return toks, gs, n_e, Tmax, cum, slot


def _prep_inputs(x, top_k_indices, expert_weights, W):
    toks, gs, n_e, Tmax, cum, slot = _route(top_k_indices, expert_weights)
    x_flat = np.asarray(x, np.float32).reshape(TOKENS, D).astype(np.float16)
    W16 = np.asarray(W, np.float32).astype(np.float16)
    in_maps = []
    for e in range(E):
        n = int(n_e[e])
        seg = slice(cum[e], cum[e] + n)
        xr = np.zeros((Tmax * P, D), np.float16)
        xr[:n] = x_flat[toks[seg]]
        # [t*128+m, c*128+p] -> [p, t, c, m]
        xg = np.ascontiguousarray(
            xr.reshape(Tmax, P, DCHUNKS, P).transpose(3, 0, 2, 1)
        ).reshape(P, Tmax * D)
        gr = np.zeros(Tmax * P, np.float32)
        gr[:n] = gs[seg]
        g_sb = np.ascontiguousarray(gr.reshape(Tmax, P).T)
        w_hw = np.ascontiguousarray(
            W16[e].reshape(DCHUNKS, P, D).transpose(1, 0, 2)
        ).reshape(P, DCHUNKS * D)
        in_maps.append({"xg": xg, "w": w_hw, "g": g_sb})
    return in_maps, Tmax, slot


def _build_program(Tmax):
    import concourse.tile as tile
    from concourse import bacc, mybir

    fp16 = mybir.dt.float16
    f32 = mybir.dt.float32

    nc = bacc.Bacc("TRN2", target_bir_lowering=False, debug=False)
    xg_d = nc.dram_tensor("xg", [P, Tmax * D], fp16, kind="ExternalInput").ap()
    w_d = nc.dram_tensor("w", [P, DCHUNKS * D], fp16, kind="ExternalInput").ap()
    g_d = nc.dram_tensor("g", [P, Tmax], f32, kind="ExternalInput").ap()
    y_d = nc.dram_tensor("y", [Tmax * P, D], fp16, kind="ExternalOutput").ap()

    with tile.TileContext(nc) as tc:
        with (
            tc.tile_pool(name="const", bufs=1) as cpool,
            tc.tile_pool(name="xp", bufs=Tmax) as xpool,
            tc.tile_pool(name="yp", bufs=3) as ypool,
            tc.tile_pool(name="ps", bufs=4, space="PSUM") as pspool,
        ):
            # Two HWDGE queues (sync SP + scalar ACT), each ~130 GB/s when both
            # stream and limited to 4 in-flight transfers. The head is
            # bandwidth-bound: W (2 MiB) + xg0/xg1 must land before tiles 0-1
            # finish, so supply is issued in half-chunk (128 KB) transfers,
            # dealt across the queues in consumption order; tiles 0-1 are
            # computed chunk-major so the PE rides the arrival stream.
            xgs = [xpool.tile([P, D], fp16, tag="xg", name="xg") for t in range(Tmax)]
            whs = [
                [cpool.tile([P, FH], fp16, name=f"w{c}h{h}") for h in range(NH)]
                for c in range(DCHUNKS)
            ]
            g_sb = cpool.tile([P, Tmax], f32)

            def wsl(c, h):  # rhs AP for chunk c, psum half h
                return whs[c][h][:]

            def kick_xg_half(ring, t, a):
                ring.dma_start(
                    xgs[t][:, a * FH : (a + 1) * FH],
                    xg_d[:, t * D + a * FH : t * D + (a + 1) * FH],
                )

            def kick_w(ring, c, h):
                ring.dma_start(
                    whs[c][h][:], w_d[:, c * D + h * FH : c * D + (h + 1) * FH]
                )

            # Supply deal-out in need order (tiles 0-2 chunk-major consumption):
            # all pieces as 128 KB half-chunks, alternating queues down the
            # need list so each chunk's halves arrive adjacently and W leads.
            # NI=3 leaves PSUM buffer 3 free, so the first serial tile never
            # waits on a DVE drain (any PE idle gap >100ns resets the p-state
            # ramp and costs ~3us of half-rate matmuls).
            NI = min(3, Tmax)  # tiles interleaved chunk-major in the head
            need = [
                ("w", 0, 0), ("w", 0, 1), ("xga", 0), ("xga", 1),
                ("w", 1, 0), ("xga", 2), ("w", 1, 1),
                ("w", 2, 0), ("w", 2, 1), ("w", 3, 0), ("w", 3, 1),
                ("w", 4, 0), ("xgb", 0), ("w", 4, 1), ("xgb", 1),
                ("w", 5, 0), ("xgb", 2), ("w", 5, 1),
                ("w", 6, 0), ("w", 6, 1), ("w", 7, 0), ("w", 7, 1),
                ("g",),
            ]
            for i, item in enumerate(need):
                ring = nc.sync if i % 2 == 0 else nc.scalar
                if item[0] == "w":
                    kick_w(ring, item[1], item[2])
                elif item[0] == "xga":
                    kick_xg_half(ring, item[1], 0)
                elif item[0] == "xgb":
                    kick_xg_half(ring, item[1], 1)
                else:
                    ring.dma_start(g_sb[:], g_d[:])
            for t in range(NI, Tmax):
                nc.scalar.dma_start(xgs[t][:], xg_d[:, t * D : (t + 1) * D])

            # PSUM tiles for the chunk-major head (all 8 banks; the pool then
            # rotates through the same 4 buffers for the serial tiles).
            psNI = [pspool.tile([P, D], f32, tag="ps", name="ps") for _ in range(NI)]

            # PE warmup on FULL-SIZE dummy matmuls (128 contraction x 512 out):
            # the clock ramp is utilization-driven, so 1-row warmups leave the
            # PE at the 1.2 GHz mid p-state and the first ~4us of real matmuls
            # run at half rate. Full-size warmups ramp to 2.4 GHz while the
            # first supply transfers are still in flight. Targets ps0
            # (complete start/stop groups, reset by the first real matmul).
            wl = cpool.tile([P, P], fp16)
            nc.vector.memset(wl[:], 1.0)
            wr = cpool.tile([P, FH], fp16)
            nc.vector.memset(wr[:], 1.0)
            for _ in range(WARMUP):
                nc.tensor.matmul(
                    psNI[0][:, 0:FH], wl[:], wr[:], start=True, stop=True
                )

            def scale_store(t, ps, last):
                """PSUM -> fp16 gate-scale -> DRAM store. The final tile's two
                halves run in PARALLEL on vector (DVE) and scalar (ACT) with
                separate queue stores, halving the drain critical path."""
                y_sb = ypool.tile([P, D], fp16, tag="y", name="y_sb")
                if not last:
                    nc.vector.tensor_scalar_mul(y_sb[:], ps[:], g_sb[:, t : t + 1])
                    nc.sync.dma_start(y_d[t * P : (t + 1) * P, :], y_sb[:])
                    return
                nc.vector.tensor_scalar_mul(
                    y_sb[:, 0:FH], ps[:, 0:FH], g_sb[:, t : t + 1]
                )
                nc.scalar.mul(y_sb[:, FH:D], ps[:, FH:D], g_sb[:, t : t + 1])
                nc.sync.dma_start(y_d[t * P : (t + 1) * P, 0:FH], y_sb[:, 0:FH])
                nc.scalar.dma_start(y_d[t * P : (t + 1) * P, FH:D], y_sb[:, FH:D])

            # Tiles 0..NI-1 chunk-major: each W half-chunk is consumed by all
            # NI tiles as soon as it lands; with NI*2*213ns of work per chunk
            # the PE outpaces the supply stream and never goes idle.
            for c in range(DCHUNKS):
                for tt in range(NI):
                    for h in range(NH):
                        nc.tensor.matmul(
                            psNI[tt][:, h * FH : (h + 1) * FH],
                            xgs[tt][:, c * P : (c + 1) * P],
                            wsl(c, h),
                            start=(c == 0),
                            stop=(c == DCHUNKS - 1),
                        )
            # Head tiles' DVE in halves: releases PSUM banks to tile NI (which
            # reuses buffer 0) half a DVE pass sooner.
            for tt in range(NI):
                y_sb = ypool.tile([P, D], fp16, tag="y", name="y_sb")
                for h in range(NH):
                    sl = slice(h * FH, (h + 1) * FH)
                    nc.vector.tensor_scalar_mul(
                        y_sb[:, sl], psNI[tt][:, sl], g_sb[:, tt : tt + 1]
                    )
                nc.sync.dma_start(y_d[tt * P : (tt + 1) * P, :], y_sb[:])

            for t in range(NI, Tmax):
                ps = pspool.tile([P, D], f32, tag="ps", name="ps")
                for c in range(DCHUNKS):
                    lhsT = xgs[t][:, c * P : (c + 1) * P]
                    for h in range(NH):
                        nc.tensor.matmul(
                            ps[:, h * FH : (h + 1) * FH],
                            lhsT,
                            wsl(c, h),
                            start=(c == 0),
                            stop=(c == DCHUNKS - 1),
                        )
                scale_store(t, ps, last=(t == Tmax - 1))
    nc.compile()
    return nc


def kernel(x, expert_weights, top_k_indices, W, b):
    from concourse.bass_utils import run_bass_kernel_spmd

    in_maps, Tmax, slot = _prep_inputs(x, top_k_indices, expert_weights, W)
    nc = _build_program(Tmax)
    res = run_bass_kernel_spmd(
        nc,
        in_maps,
        core_ids=list(range(N_CORES)),
        trace=bool(int(os.environ.get("KERNEL_TRACE", "0"))),
    )
    Y = np.concatenate(
        [res.results[e]["y"] for e in range(E)] + [np.zeros((1, D), np.float16)]
    ).astype(np.float32)
    idx = np.asarray(top_k_indices).reshape(-1, K)
    gw = np.asarray(expert_weights, np.float32).reshape(-1, K)
    b32 = np.asarray(b, np.float32)
    out = Y[slot[:, 0]] + Y[slot[:, 1]]
    out += gw[:, 0, None] * b32[idx[:, 0]]
    out += gw[:, 1, None] * b32[idx[:, 1]]
    if bool(int(os.environ.get("KERNEL_TRACE", "0"))):
        kernel.last_results = res
    return np.ascontiguousarray(out.reshape(B, S, D))


# revision 32
# speedup vs baseline: 1.2177x; 1.0018x over previous
"""MoE top-2 routing kernel for 8 Trainium2 NeuronCores — expert-parallel.

Problem: x[2,4096,1024] tokens, 8 experts W[8,1024,1024]+b[8,1024],
top-2 expert indices + gate weights per token.
out[t] = sum_k gate[t,k] * (x[t] @ W[idx[t,k]] + b[idx[t,k]])

Strategy (expert-parallel, host-side dispatch):
- E == n_cores == 8: core e owns expert e. The host routes: dedup the two
  (expert, gate) entries per token (same-expert duplicates merge, gates
  summed), groups entries by expert, and builds per-core inputs:
    xg   [128, Tmax*1024] fp16 — the expert's token rows, PE-transposed
         ([p, t*1024 + c*128 + m] = x[row t*128+m, c*128+p]), zero-padded
         to Tmax 128-row tiles.
    w    [128, 8*1024]    fp16 — W_e PE layout ([p, c*1024+f] = W_e[c*128+p, f])
    g    [128, Tmax]      f32  — gate per dispatch row ([m, t] = gate row t*128+m)
- Device: per 128-row tile, 16 accumulating fp16 matmuls (8 K-chunks x 2
  PSUM halves) -> DVE gate-scale (f32 PSUM -> fp16) -> contiguous DMA
  store. No gather/scatter ucode, no replicated W, no bias matmuls.
- Host combine: out[t] = Y[slot0[t]] + Y[slot1[t]] + g0*b[e0] + g1*b[e1]
  (slot1 -> zero row for merged/single-entry tokens); bias exact in f32.
- Load balance: per-expert entry counts are multinomial(~1920 +/- 40);
  Tmax = max_e ceil(n_e/128) == ceil(total_tiles/8) for typical draws, so
  expert-parallel matches the best possible row balance.
"""

import os
import sys

import numpy as np

for _p in ("/opt/trn_rl_repo", os.path.expanduser("~/.axon_site/_ro/trn_rl_repo")):
    if os.path.isdir(_p) and _p not in sys.path:
        sys.path.insert(0, _p)

B, S, D, E, K = 2, 4096, 1024, 8, 2
N_CORES = 8
TOKENS = B * S
P = 128
DCHUNKS = D // P  # 8
FH = 512  # psum bank half of D
NH = D // FH  # 2
WARMUP = 13


def _route(top_k_indices, expert_weights):
    """Dedup + group entries by expert.

    Returns (toks, gs, n_e, Tmax, cum, slot):
      toks/gs: token id and gate per dispatch entry, sorted by expert
      n_e[e]: entry count of expert e; cum[e]: its offset in the sort
      Tmax: per-core tile count = max_e ceil(n_e/128)
      slot[t, 0:2]: global padded-Y row of token t's entries (ZROW = none)
    """
    idx = np.asarray(top_k_indices).reshape(-1, K).astype(np.int64)
    gw = np.asarray(expert_weights).reshape(-1, K).astype(np.float32)
    dup = idx[:, 0] == idx[:, 1]
    g0 = np.where(dup, gw[:, 0] + gw[:, 1], gw[:, 0])
    keep = ~dup
    toks = np.concatenate([np.arange(TOKENS), np.arange(TOKENS)[keep]])
    exps = np.concatenate([idx[:, 0], idx[keep, 1]])
    gs = np.concatenate([g0, gw[keep, 1]])
    order = np.argsort(exps, kind="stable")
    toks, exps, gs = toks[order], exps[order], gs[order]
    n_e = np.bincount(exps, minlength=E)
    Tmax = max(1, int(np.max(-(-n_e // P))))
    cum = np.concatenate([[0], np.cumsum(n_e)])[:E]
    pos_in_e = np.arange(toks.size) - cum[exps]
    yrow = exps * (Tmax * P) + pos_in_e
    inv = np.empty_like(order)
    inv[order] = np.arange(order.size)
    ZROW = E * Tmax * P
    slot = np.full((TOKENS, 2), ZROW, np.int64)
    slot[:, 0] = yrow[inv[:TOKENS]]
    slot[keep, 1] = yrow[inv[TOKENS:]]
    return toks, gs, n_e, Tmax, cum, slot


def _prep_inputs(x, top_k_indices, expert_weights, W):
    toks, gs, n_e, Tmax, cum, slot = _route(top_k_indices, expert_weights)
    x_flat = np.asarray(x, np.float32).reshape(TOKENS, D).astype(np.float16)
    W16 = np.asarray(W, np.float32).astype(np.float16)
    in_maps = []
    for e in range(E):
        n = int(n_e[e])
        seg = slice(cum[e], cum[e] + n)
        xr = np.zeros((Tmax * P, D), np.float16)
        xr[:n] = x_flat[toks[seg]]
        # [t*128+m, c*128+p] -> [p, t, c, m]
        xg = np.ascontiguousarray(
            xr.reshape(Tmax, P, DCHUNKS, P).transpose(3, 0, 2, 1)
        ).reshape(P, Tmax * D)
        gr = np.zeros(Tmax * P, np.float32)
        gr[:n] = gs[seg]
        g_sb = np.ascontiguousarray(gr.reshape(Tmax, P).T)
        w_hw = np.ascontiguousarray(
            W16[e].reshape(DCHUNKS, P, D).transpose(1, 0, 2)
        ).reshape(P, DCHUNKS * D)
        in_maps.append({"xg": xg, "w": w_hw, "g": g_sb})
    return in_maps, Tmax, slot


def _build_program(Tmax):
    import concourse.tile as tile
    from concourse import bacc, mybir

    fp16 = mybir.dt.float16
    f32 = mybir.dt.float32

    nc = bacc.Bacc("TRN2", target_bir_lowering=False, debug=False)
    xg_d = nc.dram_tensor("xg", [P, Tmax * D], fp16, kind="ExternalInput").ap()
    w_d = nc.dram_tensor("w", [P, DCHUNKS * D], fp16, kind="ExternalInput").ap()
    g_d = nc.dram_tensor("g", [P, Tmax], f32, kind="ExternalInput").ap()
    y_d = nc.dram_tensor("y", [Tmax * P, D], fp16, kind="ExternalOutput").ap()

    with tile.TileContext(nc) as tc:
        with (
            tc.tile_pool(name="const", bufs=1) as cpool,
            tc.tile_pool(name="xp", bufs=Tmax) as xpool,
            tc.tile_pool(name="yp", bufs=3) as ypool,
            tc.tile_pool(name="ps", bufs=4, space="PSUM") as pspool,
        ):
            # Two HWDGE queues (sync SP + scalar ACT), each ~130 GB/s when both
            # stream and limited to 4 in-flight transfers. The head is
            # bandwidth-bound: W (2 MiB) + xg0/xg1 must land before tiles 0-1
            # finish, so supply is issued in half-chunk (128 KB) transfers,
            # dealt across the queues in consumption order; tiles 0-1 are
            # computed chunk-major so the PE rides the arrival stream.
            xgs = [xpool.tile([P, D], fp16, tag="xg", name="xg") for t in range(Tmax)]
            whs = [
                [cpool.tile([P, FH], fp16, name=f"w{c}h{h}") for h in range(NH)]
                for c in range(DCHUNKS)
            ]
            g_sb = cpool.tile([P, Tmax], f32)

            def wsl(c, h):  # rhs AP for chunk c, psum half h
                return whs[c][h][:]

            def kick_xg_half(ring, t, a):
                ring.dma_start(
                    xgs[t][:, a * FH : (a + 1) * FH],
                    xg_d[:, t * D + a * FH : t * D + (a + 1) * FH],
                )

            def kick_w(ring, c, h):
                ring.dma_start(
                    whs[c][h][:], w_d[:, c * D + h * FH : c * D + (h + 1) * FH]
                )

            # Supply deal-out in need order (tiles 0-2 chunk-major consumption):
            # all pieces as 128 KB half-chunks, alternating queues down the
            # need list so each chunk's halves arrive adjacently and W leads.
            # NI=3 leaves PSUM buffer 3 free, so the first serial tile never
            # waits on a DVE drain (any PE idle gap >100ns resets the p-state
            # ramp and costs ~3us of half-rate matmuls).
            NI = min(3, Tmax)  # tiles interleaved chunk-major in the head
            need = [
                ("w", 0, 0), ("w", 0, 1), ("xga", 0), ("xga", 1),
                ("w", 1, 0), ("xga", 2), ("w", 1, 1),
                ("w", 2, 0), ("w", 2, 1), ("w", 3, 0), ("w", 3, 1),
                ("w", 4, 0), ("xgb", 0), ("w", 4, 1), ("xgb", 1),
                ("w", 5, 0), ("xgb", 2), ("w", 5, 1),
                ("w", 6, 0), ("w", 6, 1), ("w", 7, 0), ("w", 7, 1),
                ("g",),
            ]
            for i, item in enumerate(need):
                ring = nc.sync if i % 2 == 0 else nc.scalar
                if item[0] == "w":
                    kick_w(ring, item[1], item[2])
                elif item[0] == "xga":
                    kick_xg_half(ring, item[1], 0)
                elif item[0] == "xgb":
                    kick_xg_half(ring, item[1], 1)
                else:
                    ring.dma_start(g_sb[:], g_d[:])
            for t in range(NI, Tmax):
                nc.scalar.dma_start(xgs[t][:], xg_d[:, t * D : (t + 1) * D])

            # PSUM tiles for the chunk-major head (all 8 banks; the pool then
            # rotates through the same 4 buffers for the serial tiles).
            psNI = [pspool.tile([P, D], f32, tag="ps", name="ps") for _ in range(NI)]

            # PE warmup on FULL-SIZE dummy matmuls (128 contraction x 512 out):
            # the clock ramp is utilization-driven, so 1-row warmups leave the
            # PE at the 1.2 GHz mid p-state and the first ~4us of real matmuls
            # run at half rate. Full-size warmups ramp to 2.4 GHz while the
            # first supply transfers are still in flight. Targets ps0
            # (complete start/stop groups, reset by the first real matmul).
            wl = cpool.tile([P, P], fp16)
            nc.vector.memset(wl[:], 1.0)
            wr = cpool.tile([P, FH], fp16)
            nc.vector.memset(wr[:], 1.0)
            for _ in range(WARMUP):
                nc.tensor.matmul(
                    psNI[0][:, 0:FH], wl[:], wr[:], start=True, stop=True
                )

            def scale_store(t, ps, last):
                """PSUM -> fp16 gate-scale -> DRAM store. The final tile's two
                halves run in PARALLEL on vector (DVE) and scalar (ACT) with
                separate queue stores, halving the drain critical path."""
                y_sb = ypool.tile([P, D], fp16, tag="y", name="y_sb")
                if not last:
                    nc.vector.tensor_scalar_mul(y_sb[:], ps[:], g_sb[:, t : t + 1])
                    nc.sync.dma_start(y_d[t * P : (t + 1) * P, :], y_sb[:])
                    return
                nc.vector.tensor_scalar_mul(
                    y_sb[:, 0:FH], ps[:, 0:FH], g_sb[:, t : t + 1]
                )
                nc.scalar.mul(y_sb[:, FH:D], ps[:, FH:D], g_sb[:, t : t + 1])
                nc.sync.dma_start(y_d[t * P : (t + 1) * P, 0:FH], y_sb[:, 0:FH])
                nc.scalar.dma_start(y_d[t * P : (t + 1) * P, FH:D], y_sb[:, FH:D])

            # Tiles 0..NI-1 chunk-major: each W half-chunk is consumed by all
            # NI tiles as soon as it lands; with NI*2*213ns of work per chunk
            # the PE outpaces the supply stream and never goes idle.
            for c in range(DCHUNKS):
                for tt in range(NI):
                    for h in range(NH):
                        nc.tensor.matmul(
                            psNI[tt][:, h * FH : (h + 1) * FH],
                            xgs[tt][:, c * P : (c + 1) * P],
                            wsl(c, h),
                            start=(c == 0),
                            stop=(c == DCHUNKS - 1),
                        )
            # Head tiles' DVE in halves: releases PSUM banks to tile NI (which
            # reuses buffer 0) half a DVE pass sooner.
            for tt in range(NI):
                y_sb = ypool.tile([P, D], fp16, tag="y", name="y_sb")
                for h in range(NH):
                    sl = slice(h * FH, (h + 1) * FH)
                    nc.vector.tensor_scalar_mul(
                        y_sb[:, sl], psNI[tt][:, sl], g_sb[:, tt : tt + 1]
                    )
                nc.sync.dma_start(y_d[tt * P : (tt + 1) * P, :], y_sb[:])

            for t in range(NI, Tmax):
                ps = pspool.tile([P, D], f32, tag="ps", name="ps")
                for c in range(DCHUNKS):
                    lhsT = xgs[t][:, c * P : (c + 1) * P]
                    for h in range(NH):
                        nc.tensor.matmul(
                            ps[:, h * FH : (h + 1) * FH],
                            lhsT,
                            wsl(c, h),
                            start=(c == 0),
                            stop=(c == DCHUNKS - 1),
                        )
                scale_store(t, ps, last=(t == Tmax - 1))
    nc.compile()
    return nc


def kernel(x, expert_weights, top_k_indices, W, b):
    from concourse.bass_utils import run_bass_kernel_spmd

    in_maps, Tmax, slot = _prep_inputs(x, top_k_indices, expert_weights, W)
    nc = _build_program(Tmax)
    res = run_bass_kernel_spmd(
        nc,
        in_maps,
        core_ids=list(range(N_CORES)),
        trace=bool(int(os.environ.get("KERNEL_TRACE", "0"))),
    )
    Y = np.concatenate(
        [res.results[e]["y"] for e in range(E)] + [np.zeros((1, D), np.float16)]
    ).astype(np.float32)
    idx = np.asarray(top_k_indices).reshape(-1, K)
    gw = np.asarray(expert_weights, np.float32).reshape(-1, K)
    b32 = np.asarray(b, np.float32)
    out = Y[slot[:, 0]] + Y[slot[:, 1]]
    out += gw[:, 0, None] * b32[idx[:, 0]]
    out += gw[:, 1, None] * b32[idx[:, 1]]
    if bool(int(os.environ.get("KERNEL_TRACE", "0"))):
        kernel.last_results = res
    return np.ascontiguousarray(out.reshape(B, S, D))


# revision 33
# speedup vs baseline: 1.2230x; 1.0044x over previous
"""MoE top-2 routing kernel for 8 Trainium2 NeuronCores — expert-parallel.

Problem: x[2,4096,1024] tokens, 8 experts W[8,1024,1024]+b[8,1024],
top-2 expert indices + gate weights per token.
out[t] = sum_k gate[t,k] * (x[t] @ W[idx[t,k]] + b[idx[t,k]])

Strategy (expert-parallel, host-side dispatch):
- E == n_cores == 8: core e owns expert e. The host routes: dedup the two
  (expert, gate) entries per token (same-expert duplicates merge, gates
  summed), groups entries by expert, and builds per-core inputs:
    xg   [128, Tmax*1024] fp16 — the expert's token rows, PE-transposed
         ([p, t*1024 + c*128 + m] = x[row t*128+m, c*128+p]), zero-padded
         to Tmax 128-row tiles.
    w    [128, 8*1024]    fp16 — W_e PE layout ([p, c*1024+f] = W_e[c*128+p, f])
    g    [128, Tmax]      f32  — gate per dispatch row ([m, t] = gate row t*128+m)
- Device: per 128-row tile, 16 accumulating fp16 matmuls (8 K-chunks x 2
  PSUM halves) -> DVE gate-scale (f32 PSUM -> fp16) -> contiguous DMA
  store. No gather/scatter ucode, no replicated W, no bias matmuls.
- Host combine: out[t] = Y[slot0[t]] + Y[slot1[t]] + g0*b[e0] + g1*b[e1]
  (slot1 -> zero row for merged/single-entry tokens); bias exact in f32.
- Load balance: per-expert entry counts are multinomial(~1920 +/- 40);
  Tmax = max_e ceil(n_e/128) == ceil(total_tiles/8) for typical draws, so
  expert-parallel matches the best possible row balance.
"""

import os
import sys

import numpy as np

for _p in ("/opt/trn_rl_repo", os.path.expanduser("~/.axon_site/_ro/trn_rl_repo")):
    if os.path.isdir(_p) and _p not in sys.path:
        sys.path.insert(0, _p)

B, S, D, E, K = 2, 4096, 1024, 8, 2
N_CORES = 8
TOKENS = B * S
P = 128
DCHUNKS = D // P  # 8
FH = 512  # psum bank half of D
NH = D // FH  # 2
WARMUP = 11


def _route(top_k_indices, expert_weights):
    """Dedup + group entries by expert.

    Returns (toks, gs, n_e, Tmax, cum, slot):
      toks/gs: token id and gate per dispatch entry, sorted by expert
      n_e[e]: entry count of expert e; cum[e]: its offset in the sort
      Tmax: per-core tile count = max_e ceil(n_e/128)
      slot[t, 0:2]: global padded-Y row of token t's entries (ZROW = none)
    """
    idx = np.asarray(top_k_indices).reshape(-1, K).astype(np.int64)
    gw = np.asarray(expert_weights).reshape(-1, K).astype(np.float32)
    dup = idx[:, 0] == idx[:, 1]
    g0 = np.where(dup, gw[:, 0] + gw[:, 1], gw[:, 0])
    keep = ~dup
    toks = np.concatenate([np.arange(TOKENS), np.arange(TOKENS)[keep]])
    exps = np.concatenate([idx[:, 0], idx[keep, 1]])
    gs = np.concatenate([g0, gw[keep, 1]])
    order = np.argsort(exps, kind="stable")
    toks, exps, gs = toks[order], exps[order], gs[order]
    n_e = np.bincount(exps, minlength=E)
    Tmax = max(1, int(np.max(-(-n_e // P))))
    cum = np.concatenate([[0], np.cumsum(n_e)])[:E]
    pos_in_e = np.arange(toks.size) - cum[exps]
    yrow = exps * (Tmax * P) + pos_in_e
    inv = np.empty_like(order)
    inv[order] = np.arange(order.size)
    ZROW = E * Tmax * P
    slot = np.full((TOKENS, 2), ZROW, np.int64)
    slot[:, 0] = yrow[inv[:TOKENS]]
    slot[keep, 1] = yrow[inv[TOKENS:]]
    return toks, gs, n_e, Tmax, cum, slot


def _prep_inputs(x, top_k_indices, expert_weights, W):
    toks, gs, n_e, Tmax, cum, slot = _route(top_k_indices, expert_weights)
    x_flat = np.asarray(x, np.float32).reshape(TOKENS, D).astype(np.float16)
    W16 = np.asarray(W, np.float32).astype(np.float16)
    in_maps = []
    for e in range(E):
        n = int(n_e[e])
        seg = slice(cum[e], cum[e] + n)
        xr = np.zeros((Tmax * P, D), np.float16)
        xr[:n] = x_flat[toks[seg]]
        # [t*128+m, c*128+p] -> [p, t, c, m]
        xg = np.ascontiguousarray(
            xr.reshape(Tmax, P, DCHUNKS, P).transpose(3, 0, 2, 1)
        ).reshape(P, Tmax * D)
        gr = np.zeros(Tmax * P, np.float32)
        gr[:n] = gs[seg]
        g_sb = np.ascontiguousarray(gr.reshape(Tmax, P).T)
        w_hw = np.ascontiguousarray(
            W16[e].reshape(DCHUNKS, P, D).transpose(1, 0, 2)
        ).reshape(P, DCHUNKS * D)
        in_maps.append({"xg": xg, "w": w_hw, "g": g_sb})
    return in_maps, Tmax, slot


def _build_program(Tmax):
    import concourse.tile as tile
    from concourse import bacc, mybir

    fp16 = mybir.dt.float16
    f32 = mybir.dt.float32

    nc = bacc.Bacc("TRN2", target_bir_lowering=False, debug=False)
    xg_d = nc.dram_tensor("xg", [P, Tmax * D], fp16, kind="ExternalInput").ap()
    w_d = nc.dram_tensor("w", [P, DCHUNKS * D], fp16, kind="ExternalInput").ap()
    g_d = nc.dram_tensor("g", [P, Tmax], f32, kind="ExternalInput").ap()
    y_d = nc.dram_tensor("y", [Tmax * P, D], fp16, kind="ExternalOutput").ap()

    with tile.TileContext(nc) as tc:
        with (
            tc.tile_pool(name="const", bufs=1) as cpool,
            tc.tile_pool(name="xp", bufs=Tmax) as xpool,
            tc.tile_pool(name="yp", bufs=3) as ypool,
            tc.tile_pool(name="ps", bufs=4, space="PSUM") as pspool,
        ):
            # Two HWDGE queues (sync SP + scalar ACT), each ~130 GB/s when both
            # stream and limited to 4 in-flight transfers. The head is
            # bandwidth-bound: W (2 MiB) + xg0/xg1 must land before tiles 0-1
            # finish, so supply is issued in half-chunk (128 KB) transfers,
            # dealt across the queues in consumption order; tiles 0-1 are
            # computed chunk-major so the PE rides the arrival stream.
            xgs = [xpool.tile([P, D], fp16, tag="xg", name="xg") for t in range(Tmax)]
            whs = [
                [cpool.tile([P, FH], fp16, name=f"w{c}h{h}") for h in range(NH)]
                for c in range(DCHUNKS)
            ]
            g_sb = cpool.tile([P, Tmax], f32)

            def wsl(c, h):  # rhs AP for chunk c, psum half h
                return whs[c][h][:]

            def kick_xg_half(ring, t, a):
                ring.dma_start(
                    xgs[t][:, a * FH : (a + 1) * FH],
                    xg_d[:, t * D + a * FH : t * D + (a + 1) * FH],
                )

            def kick_w(ring, c, h):
                ring.dma_start(
                    whs[c][h][:], w_d[:, c * D + h * FH : c * D + (h + 1) * FH]
                )

            # Supply deal-out in need order (tiles 0-2 chunk-major consumption):
            # all pieces as 128 KB half-chunks, alternating queues down the
            # need list so each chunk's halves arrive adjacently and W leads.
            # NI=3 leaves PSUM buffer 3 free, so the first serial tile never
            # waits on a DVE drain (any PE idle gap >100ns resets the p-state
            # ramp and costs ~3us of half-rate matmuls).
            NI = min(3, Tmax)  # tiles interleaved chunk-major in the head
            need = [
                ("w", 0, 0), ("w", 0, 1), ("xga", 0), ("xga", 1),
                ("w", 1, 0), ("xga", 2), ("w", 1, 1),
                ("w", 2, 0), ("w", 2, 1), ("w", 3, 0), ("w", 3, 1),
                ("w", 4, 0), ("xgb", 0), ("w", 4, 1), ("xgb", 1),
                ("w", 5, 0), ("xgb", 2), ("w", 5, 1),
                ("w", 6, 0), ("w", 6, 1), ("w", 7, 0), ("w", 7, 1),
                ("g",),
            ]
            for i, item in enumerate(need):
                ring = nc.sync if i % 2 == 0 else nc.scalar
                if item[0] == "w":
                    kick_w(ring, item[1], item[2])
                elif item[0] == "xga":
                    kick_xg_half(ring, item[1], 0)
                elif item[0] == "xgb":
                    kick_xg_half(ring, item[1], 1)
                else:
                    ring.dma_start(g_sb[:], g_d[:])
            for t in range(NI, Tmax):
                nc.scalar.dma_start(xgs[t][:], xg_d[:, t * D : (t + 1) * D])

            # PSUM tiles for the chunk-major head (all 8 banks; the pool then
            # rotates through the same 4 buffers for the serial tiles).
            psNI = [pspool.tile([P, D], f32, tag="ps", name="ps") for _ in range(NI)]

            # PE warmup on FULL-SIZE dummy matmuls (128 contraction x 512 out):
            # the clock ramp is utilization-driven, so 1-row warmups leave the
            # PE at the 1.2 GHz mid p-state and the first ~4us of real matmuls
            # run at half rate. Full-size warmups ramp to 2.4 GHz while the
            # first supply transfers are still in flight. Targets ps0
            # (complete start/stop groups, reset by the first real matmul).
            wl = cpool.tile([P, P], fp16)
            nc.vector.memset(wl[:], 1.0)
            wr = cpool.tile([P, FH], fp16)
            nc.vector.memset(wr[:], 1.0)
            for _ in range(WARMUP):
                nc.tensor.matmul(
                    psNI[0][:, 0:FH], wl[:], wr[:], start=True, stop=True
                )

            def scale_store(t, ps, last):
                """PSUM -> fp16 gate-scale -> DRAM store. The final tile's two
                halves run in PARALLEL on vector (DVE) and scalar (ACT) with
                separate queue stores, halving the drain critical path."""
                y_sb = ypool.tile([P, D], fp16, tag="y", name="y_sb")
                if not last:
                    nc.vector.tensor_scalar_mul(y_sb[:], ps[:], g_sb[:, t : t + 1])
                    nc.sync.dma_start(y_d[t * P : (t + 1) * P, :], y_sb[:])
                    return
                nc.vector.tensor_scalar_mul(
                    y_sb[:, 0:FH], ps[:, 0:FH], g_sb[:, t : t + 1]
                )
                nc.scalar.mul(y_sb[:, FH:D], ps[:, FH:D], g_sb[:, t : t + 1])
                nc.sync.dma_start(y_d[t * P : (t + 1) * P, 0:FH], y_sb[:, 0:FH])
                nc.scalar.dma_start(y_d[t * P : (t + 1) * P, FH:D], y_sb[:, FH:D])

            # Tiles 0..NI-1 chunk-major: each W half-chunk is consumed by all
            # NI tiles as soon as it lands; with NI*2*213ns of work per chunk
            # the PE outpaces the supply stream and never goes idle.
            for c in range(DCHUNKS):
                for tt in range(NI):
                    for h in range(NH):
                        nc.tensor.matmul(
                            psNI[tt][:, h * FH : (h + 1) * FH],
                            xgs[tt][:, c * P : (c + 1) * P],
                            wsl(c, h),
                            start=(c == 0),
                            stop=(c == DCHUNKS - 1),
                        )
            # Head tiles' DVE in halves: releases PSUM banks to tile NI (which
            # reuses buffer 0) half a DVE pass sooner.
            for tt in range(NI):
                y_sb = ypool.tile([P, D], fp16, tag="y", name="y_sb")
                for h in range(NH):
                    sl = slice(h * FH, (h + 1) * FH)
                    nc.vector.tensor_scalar_mul(
                        y_sb[:, sl], psNI[tt][:, sl], g_sb[:, tt : tt + 1]
                    )
                nc.sync.dma_start(y_d[tt * P : (tt + 1) * P, :], y_sb[:])

            for t in range(NI, Tmax):
                ps = pspool.tile([P, D], f32, tag="ps", name="ps")
                for c in range(DCHUNKS):
                    lhsT = xgs[t][:, c * P : (c + 1) * P]
                    for h in range(NH):
                        nc.tensor.matmul(
                            ps[:, h * FH : (h + 1) * FH],
                            lhsT,
                            wsl(c, h),
                            start=(c == 0),
                            stop=(c == DCHUNKS - 1),
                        )
                scale_store(t, ps, last=(t == Tmax - 1))
    nc.compile()
    return nc


def kernel(x, expert_weights, top_k_indices, W, b):
    from concourse.bass_utils import run_bass_kernel_spmd

    in_maps, Tmax, slot = _prep_inputs(x, top_k_indices, expert_weights, W)
    nc = _build_program(Tmax)
    res = run_bass_kernel_spmd(
        nc,
        in_maps,
        core_ids=list(range(N_CORES)),
        trace=bool(int(os.environ.get("KERNEL_TRACE", "0"))),
    )
    Y = np.concatenate(
        [res.results[e]["y"] for e in range(E)] + [np.zeros((1, D), np.float16)]
    ).astype(np.float32)
    idx = np.asarray(top_k_indices).reshape(-1, K)
    gw = np.asarray(expert_weights, np.float32).reshape(-1, K)
    b32 = np.asarray(b, np.float32)
    out = Y[slot[:, 0]] + Y[slot[:, 1]]
    out += gw[:, 0, None] * b32[idx[:, 0]]
    out += gw[:, 1, None] * b32[idx[:, 1]]
    if bool(int(os.environ.get("KERNEL_TRACE", "0"))):
        kernel.last_results = res
    return np.ascontiguousarray(out.reshape(B, S, D))
